# revision 9
# baseline (speedup 1.0000x reference)
"""Trainium2 kernel for nn_HSCR_67396626809127 (gnn_message_passing).

The reference network (fc1/fc2 -> 24-step KTD kinematic-tree recurrence ->
cam/pose/shape heads) contains no nonlinearity (dropout is identity in eval
mode), so the whole module is one affine map:

    out[157] = W @ [x(256) | init_pose(144) | init_shape(10) | init_cam(3)] + b

W [157,413] / b [157] are composed on host in float64 from the small weight
tensors (<5MB total), with the bias folded in as a constant-ones feature row
(K = 414).  The device then runs a single data-parallel matmul over the
B*T = 32768 tokens: each of the 8 cores handles 4096 tokens, reading
feature-major activation tiles (transposed on host) and writing a
feature-major output tile that the host transposes back.
"""

import numpy as np

ANCESTOR_INDEX = [[], [0], [0], [0], [0, 1], [0, 2], [0, 3], [0, 1, 4],
                  [0, 2, 5], [0, 3, 6], [0, 1, 4, 7], [0, 2, 5, 8],
                  [0, 3, 6, 9], [0, 3, 6, 9], [0, 3, 6, 9], [0, 3, 6, 9, 12],
                  [0, 3, 6, 9, 13], [0, 3, 6, 9, 14], [0, 3, 6, 9, 13, 16],
                  [0, 3, 6, 9, 14, 17], [0, 3, 6, 9, 13, 16, 18],
                  [0, 3, 6, 9, 14, 17, 19], [0, 3, 6, 9, 13, 16, 18, 20],
                  [0, 3, 6, 9, 14, 17, 19, 21]]
HID = 1024
NCORES = 8
B, T = 2048, 16
NTOK = B * T                 # 32768
TPC = NTOK // NCORES         # 4096 tokens per core
NOUT = 157                   # [cam 3 | pose 144 | shape 10]
KV = 414                     # 413 input features + ones row (bias)
TW = 1024                    # tokens per SBUF tile
NT = TPC // TW               # 4 tiles per core
MCH = [(0, 128), (128, 29)]  # output-feature chunks (psum partition dim)

_PROG = {}


def _compose_affine(fc1_w, fc1_b, fc2_w, fc2_b, decshape_w, decshape_b,
                    deccam_w, deccam_b, ktd_w, ktd_b):
    """Fold the whole network into out = v @ W.T + b, v = [x|pose|shape|cam]."""
    f8 = np.float64
    fc1_w, fc1_b = fc1_w.astype(f8), fc1_b.astype(f8)
    fc2_w, fc2_b = fc2_w.astype(f8), fc2_b.astype(f8)
    decshape_w, decshape_b = decshape_w.astype(f8), decshape_b.astype(f8)
    deccam_w, deccam_b = deccam_w.astype(f8), deccam_b.astype(f8)
    ktd_w, ktd_b = ktd_w.astype(f8), ktd_b.astype(f8)

    F1x, F1s = fc1_w[:, :256], fc1_w[:, 256:266]
    F2x, F2p = fc2_w[:, :256], fc2_w[:, 256:400]

    # KTD recurrence -> pose_out = G @ xc_pose + H @ init_pose + c
    G = np.zeros((24, 6, HID)); H = np.zeros((24, 6, 144)); c = np.zeros((24, 6))
    for j, anc in enumerate(ANCESTOR_INDEX):
        Wj = ktd_w[j]
        G[j] = Wj[:, :HID]
        off = HID
        for i in anc:
            A = Wj[:, off:off + 6]; off += 6
            G[j] += A @ G[i]
            H[j] += A @ H[i]
            c[j] += A @ c[i]
        # reference concatenates init_pose[..., j:j+6] (overlapping slice)
        H[j][:, j:j + 6] += Wj[:, off:off + 6]
        c[j] += ktd_b[j]
    G = G.reshape(144, HID); H = H.reshape(144, 144); c = c.reshape(144)

    Dp, Ds, Dc = deccam_w[:, :HID], deccam_w[:, HID:2 * HID], deccam_w[:, 2 * HID:]

    W = np.zeros((NOUT, 413)); b = np.zeros(NOUT)
    W[0:3, 0:256] = Dp @ F2x + Ds @ F1x
    W[0:3, 256:400] = Dp @ F2p
    W[0:3, 400:410] = Ds @ F1s
    W[0:3, 410:413] = Dc + np.eye(3)
    b[0:3] = Dp @ fc2_b + Ds @ fc1_b + deccam_b

    W[3:147, 0:256] = G @ F2x
    W[3:147, 256:400] = G @ F2p + H + np.eye(144)
    b[3:147] = G @ fc2_b + c

    W[147:157, 0:256] = decshape_w @ F1x
    W[147:157, 400:410] = decshape_w @ F1s + np.eye(10)
    b[147:157] = decshape_w @ fc1_b + decshape_b
    return W.astype(np.float32), b.astype(np.float32)


def _build_program():
    import concourse.bass as bass
    import concourse.tile as tile
    from concourse import bacc, mybir

    f32 = mybir.dt.float32
    nc = bacc.Bacc("TRN2", target_bir_lowering=False, debug=False,
                   num_devices=NCORES)
    # activations, feature-major: chunks 0..2 packed [128, 3, TPC], chunk 3 [30, TPC]
    vt012 = nc.declare_dram_parameter("vt012", [128, 3, TPC], f32, isOutput=False)
    vt3 = nc.declare_dram_parameter("vt3", [30, TPC], f32, isOutput=False)
    # weights packed [128, 4, NOUT]; chunk 3 rows 30..127 are zero (unused)
    wt = nc.declare_dram_parameter("wt", [128, 4, NOUT], f32, isOutput=False)
    ot = nc.declare_dram_parameter("ot", [NOUT, TPC], f32, isOutput=True)

    with tile.TileContext(nc) as tc:
        with (
            tc.tile_pool(name="wpool", bufs=1) as wpool,
            tc.tile_pool(name="rhs", bufs=3) as rpool,
            tc.tile_pool(name="outp", bufs=3) as opool,
            tc.tile_pool(name="psum", bufs=4, space=bass.MemorySpace.PSUM) as ppool,
        ):
            w = wpool.tile([128, 4, NOUT], f32, tag="w", name="w")
            nc.sync.dma_start(w[:], wt[:])

            for t in range(NT):
                tok = bass.ts(t, TW)
                r012 = rpool.tile([128, 3, TW], f32, tag="r012", name=f"r012_{t}")
                nc.sync.dma_start(r012[:], vt012[:, :, tok])
                r3 = rpool.tile([30, TW], f32, tag="r3", name=f"r3_{t}")
                nc.sync.dma_start(r3[:], vt3[:, tok])

                otiles = []
                for mi, (m0, dm) in enumerate(MCH):
                    o = opool.tile([dm, TW], f32, tag=f"o{mi}", name=f"o{mi}_{t}")
                    for h in range(TW // 512):
                        hs = bass.ts(h, 512)
                        ps = ppool.tile([dm, 512], f32, tag=f"ps{mi}",
                                        name=f"ps{mi}_{t}_{h}")
                        for k in range(4):
                            if k < 3:
                                lhsT, rhs = w[:, k, m0:m0 + dm], r012[:, k, hs]
                            else:
                                lhsT, rhs = w[0:30, 3, m0:m0 + dm], r3[:, hs]
                            nc.tensor.matmul(ps[:], lhsT, rhs,
                                             start=(k == 0), stop=(k == 3))
                        nc.vector.tensor_copy(o[:, hs], ps[:])
                    otiles.append((m0, dm, o))

                for m0, dm, o in otiles:
                    nc.sync.dma_start(ot[m0:m0 + dm, tok], o[:])
    nc.compile()
    return nc


def _get_program():
    if "nc" not in _PROG:
        _PROG["nc"] = _build_program()
    return _PROG["nc"]


def _make_in_maps(x, init_pose, init_shape, init_cam, fc1_w, fc1_b, fc2_w,
                  fc2_b, decshape_w, decshape_b, deccam_w, deccam_b, ktd_w,
                  ktd_b):
    x = np.asarray(x, dtype=np.float32)
    init_pose = np.asarray(init_pose, dtype=np.float32)
    init_shape = np.asarray(init_shape, dtype=np.float32)
    init_cam = np.asarray(init_cam, dtype=np.float32)

    W, b = _compose_affine(
        np.asarray(fc1_w), np.asarray(fc1_b), np.asarray(fc2_w),
        np.asarray(fc2_b), np.asarray(decshape_w), np.asarray(decshape_b),
        np.asarray(deccam_w), np.asarray(deccam_b), np.asarray(ktd_w),
        np.asarray(ktd_b))
    # augment with bias column; device weight layout is [128, 4, 157]
    # (partition p, k-chunk, out-feature), chunk 3 zero-padded past row 30
    W_aug = np.concatenate([W, b[:, None]], axis=1)        # [157, 414]
    wtk = W_aug.T                                           # [414, 157]
    wt = np.zeros((4, 128, NOUT), np.float32)
    wt[0:3] = wtk[0:384].reshape(3, 128, NOUT)
    wt[3, 0:30] = wtk[384:414]
    wt = np.ascontiguousarray(wt.transpose(1, 0, 2))        # [128, 4, 157]

    xs = x.reshape(NCORES, TPC, 256)
    ps = init_pose.reshape(NCORES, TPC, 144)
    ss = init_shape.reshape(NCORES, TPC, 10)
    cs = init_cam.reshape(NCORES, TPC, 3)

    in_maps = []
    for i in range(NCORES):
        v = np.empty((KV, TPC), np.float32)                 # feature-major shard
        v[0:256] = xs[i].T
        v[256:400] = ps[i].T
        v[400:410] = ss[i].T
        v[410:413] = cs[i].T
        v[413] = 1.0
        in_maps.append({
            "vt012": np.ascontiguousarray(
                v[0:384].reshape(3, 128, TPC).transpose(1, 0, 2)),
            "vt3": np.ascontiguousarray(v[384:414]),
            "wt": wt,
        })
    return in_maps


def _assemble(results):
    out_t = np.empty((NOUT, NTOK), np.float32)
    for i in range(NCORES):
        out_t[:, i * TPC:(i + 1) * TPC] = results[i]["ot"]
    return np.ascontiguousarray(out_t.T)


def kernel(x, init_pose, init_shape, init_cam, fc1_w, fc1_b, fc2_w, fc2_b,
           decshape_w, decshape_b, deccam_w, deccam_b, ktd_w, ktd_b):
    from concourse.bass_utils import run_bass_kernel_spmd

    in_maps = _make_in_maps(x, init_pose, init_shape, init_cam, fc1_w, fc1_b,
                            fc2_w, fc2_b, decshape_w, decshape_b, deccam_w,
                            deccam_b, ktd_w, ktd_b)
    nc = _get_program()
    res = run_bass_kernel_spmd(nc, in_maps, list(range(NCORES)))
    return _assemble(res.results)


# revision 14
# speedup vs baseline: 1.2490x; 1.2490x over previous
"""Trainium2 kernel for nn_HSCR_67396626809127 (gnn_message_passing).

The reference network (fc1/fc2 -> 24-step KTD kinematic-tree recurrence ->
cam/pose/shape heads) contains no nonlinearity (dropout is identity in eval
mode), so the whole module is one affine map:

    out[157] = W @ [x(256) | init_pose(144) | init_shape(10) | init_cam(3)] + b

W [157,413] / b [157] are composed on host in float64 from the small weight
tensors (<5MB total), with the bias folded in as a constant-ones feature row
(K = 414).  The device then runs a single data-parallel matmul over the
B*T = 32768 tokens: each of the 8 cores handles 4096 tokens, reading
feature-major activation tiles (transposed on host) and writing a
feature-major output tile that the host transposes back.
"""

import numpy as np

ANCESTOR_INDEX = [[], [0], [0], [0], [0, 1], [0, 2], [0, 3], [0, 1, 4],
                  [0, 2, 5], [0, 3, 6], [0, 1, 4, 7], [0, 2, 5, 8],
                  [0, 3, 6, 9], [0, 3, 6, 9], [0, 3, 6, 9], [0, 3, 6, 9, 12],
                  [0, 3, 6, 9, 13], [0, 3, 6, 9, 14], [0, 3, 6, 9, 13, 16],
                  [0, 3, 6, 9, 14, 17], [0, 3, 6, 9, 13, 16, 18],
                  [0, 3, 6, 9, 14, 17, 19], [0, 3, 6, 9, 13, 16, 18, 20],
                  [0, 3, 6, 9, 14, 17, 19, 21]]
HID = 1024
NCORES = 8
B, T = 2048, 16
NTOK = B * T                 # 32768
TPC = NTOK // NCORES         # 4096 tokens per core
NOUT = 157                   # [cam 3 | pose 144 | shape 10]
KV = 414                     # 413 input features + ones row (bias)
TW = 1024                    # tokens per SBUF tile
NT = TPC // TW               # 4 tiles per core
MCH = [(0, 128), (128, 29)]  # output-feature chunks (psum partition dim)

_PROG = {}


def _compose_affine(fc1_w, fc1_b, fc2_w, fc2_b, decshape_w, decshape_b,
                    deccam_w, deccam_b, ktd_w, ktd_b):
    """Fold the whole network into out = v @ W.T + b, v = [x|pose|shape|cam]."""
    f8 = np.float64
    fc1_w, fc1_b = fc1_w.astype(f8), fc1_b.astype(f8)
    fc2_w, fc2_b = fc2_w.astype(f8), fc2_b.astype(f8)
    decshape_w, decshape_b = decshape_w.astype(f8), decshape_b.astype(f8)
    deccam_w, deccam_b = deccam_w.astype(f8), deccam_b.astype(f8)
    ktd_w, ktd_b = ktd_w.astype(f8), ktd_b.astype(f8)

    F1x, F1s = fc1_w[:, :256], fc1_w[:, 256:266]
    F2x, F2p = fc2_w[:, :256], fc2_w[:, 256:400]

    # KTD recurrence -> pose_out = G @ xc_pose + H @ init_pose + c
    G = np.zeros((24, 6, HID)); H = np.zeros((24, 6, 144)); c = np.zeros((24, 6))
    for j, anc in enumerate(ANCESTOR_INDEX):
        Wj = ktd_w[j]
        G[j] = Wj[:, :HID]
        off = HID
        for i in anc:
            A = Wj[:, off:off + 6]; off += 6
            G[j] += A @ G[i]
            H[j] += A @ H[i]
            c[j] += A @ c[i]
        # reference concatenates init_pose[..., j:j+6] (overlapping slice)
        H[j][:, j:j + 6] += Wj[:, off:off + 6]
        c[j] += ktd_b[j]
    G = G.reshape(144, HID); H = H.reshape(144, 144); c = c.reshape(144)

    Dp, Ds, Dc = deccam_w[:, :HID], deccam_w[:, HID:2 * HID], deccam_w[:, 2 * HID:]

    W = np.zeros((NOUT, 413)); b = np.zeros(NOUT)
    W[0:3, 0:256] = Dp @ F2x + Ds @ F1x
    W[0:3, 256:400] = Dp @ F2p
    W[0:3, 400:410] = Ds @ F1s
    W[0:3, 410:413] = Dc + np.eye(3)
    b[0:3] = Dp @ fc2_b + Ds @ fc1_b + deccam_b

    W[3:147, 0:256] = G @ F2x
    W[3:147, 256:400] = G @ F2p + H + np.eye(144)
    b[3:147] = G @ fc2_b + c

    W[147:157, 0:256] = decshape_w @ F1x
    W[147:157, 400:410] = decshape_w @ F1s + np.eye(10)
    b[147:157] = decshape_w @ fc1_b + decshape_b
    return W.astype(np.float32), b.astype(np.float32)


def _build_program():
    import concourse.bass as bass
    import concourse.tile as tile
    from concourse import bacc, mybir

    f32 = mybir.dt.float32
    f32r = mybir.dt.float32r
    nc = bacc.Bacc("TRN2", target_bir_lowering=False, debug=False,
                   num_devices=NCORES)
    # activations, feature-major: chunks 0..2 packed [128, 3, TPC], chunk 3 [30, TPC]
    # float32r end-to-end: same 4-byte data, PE streams 1 cycle/row vs 4 for f32
    vt012 = nc.declare_dram_parameter("vt012", [128, 3, TPC], f32r, isOutput=False)
    vt3 = nc.declare_dram_parameter("vt3", [30, TPC], f32r, isOutput=False)
    # weights packed [128, 4, NOUT]; chunk 3 rows 30..127 are zero (unused)
    wt = nc.declare_dram_parameter("wt", [128, 4, NOUT], f32r, isOutput=False)
    ot = nc.declare_dram_parameter("ot", [NOUT, TPC], f32, isOutput=True)

    with tile.TileContext(nc) as tc:
        with (
            tc.tile_pool(name="wpool", bufs=1) as wpool,
            tc.tile_pool(name="rhs", bufs=4) as rpool,
            tc.tile_pool(name="outp", bufs=3) as opool,
            tc.tile_pool(name="psum", bufs=4, space=bass.MemorySpace.PSUM) as ppool,
        ):
            w = wpool.tile([128, 4, NOUT], f32r, tag="w", name="w")
            nc.sync.dma_start(w[:], wt[:])

            for t in range(NT):
                tok = bass.ts(t, TW)
                r012 = rpool.tile([128, 3, TW], f32r, tag="r012", name=f"r012_{t}")
                nc.sync.dma_start(r012[:], vt012[:, :, tok])
                r3 = rpool.tile([30, TW], f32r, tag="r3", name=f"r3_{t}")
                nc.sync.dma_start(r3[:], vt3[:, tok])

                otiles = []
                for mi, (m0, dm) in enumerate(MCH):
                    o = opool.tile([dm, TW], f32, tag=f"o{mi}", name=f"o{mi}_{t}")
                    for h in range(TW // 512):
                        hs = bass.ts(h, 512)
                        ps = ppool.tile([dm, 512], f32, tag=f"ps{mi}",
                                        name=f"ps{mi}_{t}_{h}")
                        for k in range(4):
                            if k < 3:
                                lhsT, rhs = w[:, k, m0:m0 + dm], r012[:, k, hs]
                            else:
                                lhsT, rhs = w[0:30, 3, m0:m0 + dm], r3[:, hs]
                            # float32r streams at 1 cycle/row for N>=256
                            # (plain fp32 pays 4x); same 4-byte data
                            nc.tensor.matmul(ps[:], lhsT, rhs,
                                             start=(k == 0), stop=(k == 3))
                        nc.vector.tensor_copy(o[:, hs], ps[:])
                    otiles.append((m0, dm, o))

                for m0, dm, o in otiles:
                    nc.sync.dma_start(ot[m0:m0 + dm, tok], o[:])
    nc.compile()
    return nc


def _get_program():
    if "nc" not in _PROG:
        _PROG["nc"] = _build_program()
    return _PROG["nc"]


def _make_in_maps(x, init_pose, init_shape, init_cam, fc1_w, fc1_b, fc2_w,
                  fc2_b, decshape_w, decshape_b, deccam_w, deccam_b, ktd_w,
                  ktd_b):
    x = np.asarray(x, dtype=np.float32)
    init_pose = np.asarray(init_pose, dtype=np.float32)
    init_shape = np.asarray(init_shape, dtype=np.float32)
    init_cam = np.asarray(init_cam, dtype=np.float32)

    W, b = _compose_affine(
        np.asarray(fc1_w), np.asarray(fc1_b), np.asarray(fc2_w),
        np.asarray(fc2_b), np.asarray(decshape_w), np.asarray(decshape_b),
        np.asarray(deccam_w), np.asarray(deccam_b), np.asarray(ktd_w),
        np.asarray(ktd_b))
    # augment with bias column; device weight layout is [128, 4, 157]
    # (partition p, k-chunk, out-feature), chunk 3 zero-padded past row 30
    W_aug = np.concatenate([W, b[:, None]], axis=1)        # [157, 414]
    wtk = W_aug.T                                           # [414, 157]
    wt = np.zeros((4, 128, NOUT), np.float32)
    wt[0:3] = wtk[0:384].reshape(3, 128, NOUT)
    wt[3, 0:30] = wtk[384:414]
    wt = np.ascontiguousarray(wt.transpose(1, 0, 2))        # [128, 4, 157]

    xs = x.reshape(NCORES, TPC, 256)
    ps = init_pose.reshape(NCORES, TPC, 144)
    ss = init_shape.reshape(NCORES, TPC, 10)
    cs = init_cam.reshape(NCORES, TPC, 3)

    in_maps = []
    for i in range(NCORES):
        v = np.empty((KV, TPC), np.float32)                 # feature-major shard
        v[0:256] = xs[i].T
        v[256:400] = ps[i].T
        v[400:410] = ss[i].T
        v[410:413] = cs[i].T
        v[413] = 1.0
        in_maps.append({
            "vt012": np.ascontiguousarray(
                v[0:384].reshape(3, 128, TPC).transpose(1, 0, 2)),
            "vt3": np.ascontiguousarray(v[384:414]),
            "wt": wt,
        })
    return in_maps


def _assemble(results):
    out_t = np.empty((NOUT, NTOK), np.float32)
    for i in range(NCORES):
        out_t[:, i * TPC:(i + 1) * TPC] = results[i]["ot"]
    return np.ascontiguousarray(out_t.T)


def kernel(x, init_pose, init_shape, init_cam, fc1_w, fc1_b, fc2_w, fc2_b,
           decshape_w, decshape_b, deccam_w, deccam_b, ktd_w, ktd_b):
    from concourse.bass_utils import run_bass_kernel_spmd

    in_maps = _make_in_maps(x, init_pose, init_shape, init_cam, fc1_w, fc1_b,
                            fc2_w, fc2_b, decshape_w, decshape_b, deccam_w,
                            deccam_b, ktd_w, ktd_b)
    nc = _get_program()
    res = run_bass_kernel_spmd(nc, in_maps, list(range(NCORES)))
    return _assemble(res.results)
